# revision 12
# baseline (speedup 1.0000x reference)
"""Trainium2 Bass kernel for nn_Composer (gnn_message_passing).

Math (exact reformulation of the reference):
  out[b,s1,:] = (heads[b,s1]==0) * ( base + sum_{s2: heads[b,s2]==s1} w[s2]*(t_on[b,s2]-t_off) )
  t_on[b,s2]  = tanh(u[b,s2] + bc),  u[b,s2,o] = tok[b,s2] @ Wc[o] @ tanh(tok[b,s2])
  t_off       = tanh(bc),  base = t_off*sum(w) + br

Only rows s2 whose head lands on a row with head==0 contribute to the output,
so u is needed for a handful of rows (R ~ 4-16 of 4096). The unavoidable cost
is streaming the bilinear weight Wc once; with the loose output tolerance Wc
is quantized host-side to fp8e4 (scaled by 32, rescaled in the reduce), which
cuts HBM traffic 4x vs f32 (7.08 MB/core). Sharding: Wc split over the output
dim O=384 across 8 cores (48 channels each). Channels are processed in PAIRS:
two col-tiled matmuls (tile_position (0,0)/(0,64)) put channel 2j in PSUM
partitions 0-63 and channel 2j+1 in 64-127 concurrently, so the fused
multiply+reduce against dep on the vector engine handles two channels per op.
The device outputs the raw bilinear values u; the tiny tanh/scale epilogue
(24 values/partition) runs on the host, keeping the device-side tail to one
reduce and one small DMA. The host does index selection, fp8/bf16 conversion,
sharding, and the final scatter of the ~R result vectors into the zero output.
"""
import numpy as np
import ml_dtypes

import concourse.bass as bass
import concourse.bacc as bacc
import concourse.mybir as mybir
from concourse.tile import TileContext
from concourse.bass_utils import run_bass_kernel_spmd

F32 = mybir.dt.float32
BF16 = mybir.dt.bfloat16
FP8 = mybir.dt.float8e4

B, S, D = 8, 512, 384
NCORES = 8
OC = D // NCORES          # output channels per core = 48
NPAIR = OC // 2           # channel pairs per core = 24
DC = D // 128             # contraction chunks = 3
R_MAX = 64                # padded selected-row capacity per device run
SCALE = 32.0              # Wc is stored as fp8(SCALE*Wc); undone in the reduce
# Wc transfer group sizes in output channels (even, so pairs never straddle a
# group): small head groups so compute starts early, big middle groups for DMA
# efficiency, small tail groups so the final DMA->compute->output is short.
# Groups alternate between the two HWDGE rings so both stream concurrently at
# the aggregate HBM rate.
GROUP_SIZES = [2, 2, 4, 6, 8, 8, 8, 6, 2, 2]
assert sum(GROUP_SIZES) == OC and all(g % 2 == 0 for g in GROUP_SIZES)
N_GRP = len(GROUP_SIZES)
# u output segments (in pair columns): flushed after pairs 11, 22, 23 so only
# the last single-pair segment's DMA completion sits on the critical tail
U_SEGS = [(0, 12), (12, 23), (23, 24)]

_nc_cache = {}


def _build_nc():
    if "nc" in _nc_cache:
        return _nc_cache["nc"]
    nc = bacc.Bacc("TRN2", target_bir_lowering=False, debug=False)
    wc_d = nc.dram_tensor("wc", [128, OC * DC * 384], FP8, kind="ExternalInput")
    tokT_d = nc.dram_tensor("tokT", [128, DC * R_MAX], BF16, kind="ExternalInput")
    dep_d = nc.dram_tensor("dep", [128, D], F32, kind="ExternalInput")
    u_d = nc.dram_tensor("u", [128, NPAIR], F32, kind="ExternalOutput")

    OP = mybir.AluOpType

    offs = [sum(GROUP_SIZES[:g]) for g in range(N_GRP)]

    with TileContext(nc) as tc:
        with (
            tc.tile_pool(name="const", bufs=1) as cp,
            tc.tile_pool(name="wcp", bufs=2 * N_GRP) as wcp,
            tc.tile_pool(name="zp", bufs=8) as zp,
            tc.tile_pool(name="pp", bufs=4, space="PSUM") as pp,
            tc.tile_pool(name="ppw", bufs=1, space="PSUM") as ppw,
        ):
            # dep is first needed by the first reduce: it leads the scalar
            # HWDGE ring; wc group-halves then alternate across both rings.
            dep_sb = cp.tile([128, D], F32)
            nc.scalar.dma_start(out=dep_sb[:], in_=dep_d[:])

            wts = []
            tokT_sb = None
            for g in range(N_GRP):
                wt = wcp.tile([128, GROUP_SIZES[g] * DC * 384], FP8,
                              tag="wc", name=f"wt{g}")
                eng = nc.sync if g % 2 == 0 else nc.scalar
                eng.dma_start(
                    out=wt[:],
                    in_=wc_d[:, offs[g] * DC * 384:
                             (offs[g] + GROUP_SIZES[g]) * DC * 384])
                wts.append(wt)
                if g == 0:
                    # tokT rides the SP ring right behind wc group 0
                    tokT_sb = cp.tile([128, DC * R_MAX], BF16)
                    nc.sync.dma_start(out=tokT_sb[:], in_=tokT_d[:])

            # PE warm-up: ~3.8us of dummy matmuls on zeroed scratch during the
            # preamble + DMA-ramp dead window flips the HAM clock gate to 8/8
            # before the first real matmul, so real pairs never run at 1.2 GHz
            # and the PE never falls behind the wc stream.
            wl = cp.tile([128, R_MAX], BF16, tag="wl", name="wl")
            wr = cp.tile([128, 384], FP8, tag="wr", name="wr")
            nc.gpsimd.memset(wl[:], 0)
            nc.gpsimd.memset(wr[:], 0)
            wps = ppw.tile([128, 384], F32, tag="wps")
            for _ in range(12):
                nc.tensor.matmul(wps[0:64, :], lhsT=wl[:], rhs=wr[:],
                                 start=True, stop=True)

            useg = [cp.tile([128, hi - lo], F32, tag=f"u{lo}", name=f"u{lo}")
                    for lo, hi in U_SEGS]

            pair = 0
            for g in range(N_GRP):
                wt = wts[g]
                for i in range(GROUP_SIZES[g] // 2):
                    lA, lB = 2 * i, 2 * i + 1          # local channel idx
                    ps = pp.tile([128, 384], F32, tag="ps")
                    # interleave the two col-tiles so both array halves
                    # stream their moving operand concurrently
                    for c in range(DC):
                        nc.tensor.matmul(
                            ps[0:64, :],
                            lhsT=tokT_sb[:, c * R_MAX:(c + 1) * R_MAX],
                            rhs=wt[:, (lA * DC + c) * 384:(lA * DC + c + 1) * 384],
                            start=(c == 0), stop=(c == DC - 1))
                        nc.tensor.matmul(
                            ps[64:128, :],
                            lhsT=tokT_sb[:, c * R_MAX:(c + 1) * R_MAX],
                            rhs=wt[:, (lB * DC + c) * 384:(lB * DC + c + 1) * 384],
                            start=(c == 0), stop=(c == DC - 1))
                    si, col = ((0, pair) if pair < 12
                               else (1, pair - 12) if pair < 23
                               else (2, pair - 23))
                    z = zp.tile([128, 384], F32, tag="z")
                    # u[:, pair] = sum_e (ps/SCALE) * dep  (fused mul+reduce)
                    nc.vector.scalar_tensor_tensor(
                        out=z[:], in0=ps[:], scalar=1.0 / SCALE, in1=dep_sb[:],
                        op0=OP.mult, op1=OP.mult,
                        accum_out=useg[si][:, col:col + 1])
                    # pacing: the warm PE consumes a pair in ~490ns but the
                    # stream delivers one per ~830ns. Two scratch matmuls per
                    # pair absorb the deficit so the PE never idles long
                    # enough for the HAM clock gate to re-throttle it to
                    # half rate. Skipped near the end to keep the tail short.
                    if pair < NPAIR - 3:
                        for _ in range(2):
                            nc.tensor.matmul(wps[0:64, :], lhsT=wl[:],
                                             rhs=wr[:], start=True, stop=True)
                    pair += 1
                    for k, (lo, hi) in enumerate(U_SEGS):
                        if pair == hi:
                            nc.scalar.dma_start(out=u_d[:, lo:hi],
                                                in_=useg[k][:])

    nc.compile()
    _nc_cache["nc"] = nc
    return nc


def _shard_wc(Wc):
    """Per-core Wc layout: [128(p), OC*DC*384] fp8e4 of SCALE*Wc, with
    d = c*128 + p and free index f = (o_local*DC + c)*384 + e."""
    shards = []
    for k in range(NCORES):
        wck = Wc[k * OC:(k + 1) * OC]                  # [48, 384, 384]
        wck = wck.reshape(OC, DC, 128, 384)            # o, c, p, e
        wck = wck.transpose(2, 0, 1, 3)                # p, o, c, e
        q = (wck.reshape(128, OC * DC * 384) * SCALE).astype(
            ml_dtypes.float8_e4m3)
        shards.append(np.ascontiguousarray(q))
    return shards


def run_device(in_maps, trace=False, tmpdir=None):
    nc = _build_nc()
    return run_bass_kernel_spmd(nc, in_maps, list(range(NCORES)),
                                trace=trace, tmpdir=tmpdir)


def _make_in_maps(tok_sel, w_sel, wc_shards, bc):
    """tok_sel [R_MAX, D] f32 (w_sel/bc handled host-side post-epilogue)."""
    # tokT[p, c*R_MAX + r] = tok_sel[r, c*128 + p]
    tokT = np.ascontiguousarray(
        tok_sel.T.reshape(DC, 128, R_MAX).transpose(1, 0, 2)
    ).reshape(128, DC * R_MAX).astype(ml_dtypes.bfloat16)
    dep2 = np.tanh(np.concatenate([tok_sel, tok_sel], axis=0)).astype(np.float32)
    return [{"wc": wc_shards[k], "tokT": tokT, "dep": dep2}
            for k in range(NCORES)]


def kernel(**inputs):
    tokens = np.asarray(inputs["tokens"])
    heads = np.asarray(inputs["dep_heads"])
    tok_table = np.asarray(inputs["tok_table"], dtype=np.float32)
    Wc = np.asarray(inputs["Wc"], dtype=np.float32)
    bc = np.asarray(inputs["bc"], dtype=np.float32)
    Wr = np.asarray(inputs["Wr"], dtype=np.float32)
    br = np.asarray(inputs["br"], dtype=np.float32)
    assert tokens.shape == (B, S) and Wc.shape == (D, D, D)

    # host index selection: rows that can reach an unmasked (head==0) output row
    zs = [np.nonzero(heads[b] == 0)[0] for b in range(B)]
    sel = [(b, int(s2), int(heads[b, s2]))
           for b in range(B)
           for s2 in np.nonzero(np.isin(heads[b], zs[b]))[0]]
    R = len(sel)

    wc_shards = _shard_wc(Wc)
    w_full = Wr[0]
    toff = np.tanh(bc)

    contribs = []
    for lo in range(0, max(R, 1), R_MAX):
        chunk = sel[lo:lo + R_MAX]
        tok_sel = np.zeros((R_MAX, D), dtype=np.float32)
        w_sel = np.zeros(R_MAX, dtype=np.float32)
        for i, (b, s2, _dest) in enumerate(chunk):
            tok_sel[i] = tok_table[tokens[b, s2]]
            w_sel[i] = w_full[s2]
        res = run_device(_make_in_maps(tok_sel, w_sel, wc_shards, bc)).results
        # unscramble pair layout: rows 0-63 = even channels, 64-127 = odd
        parts = []
        for k in range(NCORES):
            uk = np.empty((R_MAX, OC), dtype=np.float32)
            uk[:, 0::2] = res[k]["u"][0:R_MAX]
            uk[:, 1::2] = res[k]["u"][R_MAX:2 * R_MAX]
            parts.append(uk)
        u = np.concatenate(parts, axis=1)              # [R_MAX, D]
        contribs.append(w_sel[:, None] * (np.tanh(u + bc[None, :])
                                          - toff[None, :]))

    base = (toff * w_full.sum() + br[0]).astype(np.float32)
    out = np.zeros((B, S, D), dtype=np.float32)
    for b in range(B):
        out[b, zs[b]] = base
    for i, (b, _s2, dest) in enumerate(sel):
        out[b, dest] += contribs[i // R_MAX][i % R_MAX]
    return out


# revision 14
# speedup vs baseline: 1.2846x; 1.2846x over previous
"""Trainium2 Bass kernel for nn_Composer (gnn_message_passing).

Math (exact reformulation of the reference):
  out[b,s1,:] = (heads[b,s1]==0) * ( base + sum_{s2: heads[b,s2]==s1} w[s2]*(t_on[b,s2]-t_off) )
  t_on[b,s2]  = tanh(u[b,s2] + bc),  u[b,s2,o] = tok[b,s2] @ Wc[o] @ tanh(tok[b,s2])
  t_off       = tanh(bc),  base = t_off*sum(w) + br

Only rows s2 whose head lands on a row with head==0 contribute to the output,
so u is needed for a handful of rows (R ~ 4-16 of 4096). The unavoidable cost
is streaming the bilinear weight Wc once; with the loose output tolerance Wc
is quantized host-side to fp8e4 (scaled by 32, rescaled in the reduce), which
cuts HBM traffic 4x vs f32 (7.08 MB/core). Sharding: Wc split over the output
dim O=384 across 8 cores (48 channels each). Channels are processed in PAIRS:
two col-tiled matmuls (tile_position (0,0)/(0,64)) put channel 2j in PSUM
partitions 0-63 and channel 2j+1 in 64-127 concurrently, so the fused
multiply+reduce against dep on the vector engine handles two channels per op.
The device outputs the raw bilinear values u; the tiny tanh/scale epilogue
(24 values/partition) runs on the host, keeping the device-side tail to one
reduce and one small DMA. The host does index selection, fp8/bf16 conversion,
sharding, and the final scatter of the ~R result vectors into the zero output.
"""
import numpy as np
import ml_dtypes

import concourse.bass as bass
import concourse.bacc as bacc
import concourse.mybir as mybir
from concourse.tile import TileContext
from concourse.bass_utils import run_bass_kernel_spmd

F32 = mybir.dt.float32
BF16 = mybir.dt.bfloat16
FP8 = mybir.dt.float8e4

B, S, D = 8, 512, 384
NCORES = 8
OC = D // NCORES          # output channels per core = 48
NPAIR = OC // 2           # channel pairs per core = 24
DC = D // 128             # contraction chunks = 3
R_MAX = 64                # padded selected-row capacity per device run
SCALE = 32.0              # Wc is stored as fp8(SCALE*Wc); undone in the reduce
# Wc transfer groups: one channel-pair per group, alternating between the two
# HWDGE rings. Each ring then delivers a group every ~1.7us and the aggregate
# stream hands the consumer one pair per ~830ns -- just above the warm PE's
# ~490ns/pair -- so PE idle accrues in ~340ns slivers that never span a HAM
# activity window (a >=3.4us PE idle would re-throttle the clock to half rate).
GROUP_SIZES = [2] * NPAIR
assert sum(GROUP_SIZES) == OC and all(g % 2 == 0 for g in GROUP_SIZES)
N_GRP = len(GROUP_SIZES)
# u output segments (in pair columns): flushed after pairs 11, 22, 23 so only
# the last single-pair segment's DMA completion sits on the critical tail
U_SEGS = [(0, 12), (12, 23), (23, 24)]

_nc_cache = {}


def _build_nc():
    if "nc" in _nc_cache:
        return _nc_cache["nc"]
    nc = bacc.Bacc("TRN2", target_bir_lowering=False, debug=False)
    wc_d = nc.dram_tensor("wc", [128, OC * DC * 384], FP8, kind="ExternalInput")
    tokT_d = nc.dram_tensor("tokT", [128, DC * R_MAX], BF16, kind="ExternalInput")
    dep_d = nc.dram_tensor("dep", [128, D], F32, kind="ExternalInput")
    u_d = nc.dram_tensor("u", [128, NPAIR], F32, kind="ExternalOutput")

    OP = mybir.AluOpType

    offs = [sum(GROUP_SIZES[:g]) for g in range(N_GRP)]

    with TileContext(nc) as tc:
        with (
            tc.tile_pool(name="const", bufs=1) as cp,
            tc.tile_pool(name="wcp", bufs=2 * N_GRP) as wcp,
            tc.tile_pool(name="zp", bufs=8) as zp,
            tc.tile_pool(name="pp", bufs=4, space="PSUM") as pp,
            tc.tile_pool(name="ppw", bufs=1, space="PSUM") as ppw,
        ):
            # dep is first needed by the first reduce: it leads the scalar
            # HWDGE ring; wc group-halves then alternate across both rings.
            dep_sb = cp.tile([128, D], F32)
            nc.scalar.dma_start(out=dep_sb[:], in_=dep_d[:])

            wts = []
            tokT_sb = None
            for g in range(N_GRP):
                wt = wcp.tile([128, GROUP_SIZES[g] * DC * 384], FP8,
                              tag="wc", name=f"wt{g}")
                eng = nc.sync if g % 2 == 0 else nc.scalar
                eng.dma_start(
                    out=wt[:],
                    in_=wc_d[:, offs[g] * DC * 384:
                             (offs[g] + GROUP_SIZES[g]) * DC * 384])
                wts.append(wt)
                if g == 0:
                    # tokT rides the SP ring right behind wc group 0
                    tokT_sb = cp.tile([128, DC * R_MAX], BF16)
                    nc.sync.dma_start(out=tokT_sb[:], in_=tokT_d[:])

            # PE warm-up: ~3.8us of dummy matmuls on zeroed scratch during the
            # preamble + DMA-ramp dead window flips the HAM clock gate to 8/8
            # before the first real matmul, so real pairs never run at 1.2 GHz
            # and the PE never falls behind the wc stream.
            wl = cp.tile([128, R_MAX], BF16, tag="wl", name="wl")
            wr = cp.tile([128, 384], FP8, tag="wr", name="wr")
            nc.gpsimd.memset(wl[:], 0)
            nc.gpsimd.memset(wr[:], 0)
            wps = ppw.tile([128, 384], F32, tag="wps")
            for _ in range(12):
                nc.tensor.matmul(wps[0:64, :], lhsT=wl[:], rhs=wr[:],
                                 start=True, stop=True)

            useg = [cp.tile([128, hi - lo], F32, tag=f"u{lo}", name=f"u{lo}")
                    for lo, hi in U_SEGS]

            pair = 0
            for g in range(N_GRP):
                wt = wts[g]
                for i in range(GROUP_SIZES[g] // 2):
                    lA, lB = 2 * i, 2 * i + 1          # local channel idx
                    ps = pp.tile([128, 384], F32, tag="ps")
                    # interleave the two col-tiles so both array halves
                    # stream their moving operand concurrently
                    for c in range(DC):
                        nc.tensor.matmul(
                            ps[0:64, :],
                            lhsT=tokT_sb[:, c * R_MAX:(c + 1) * R_MAX],
                            rhs=wt[:, (lA * DC + c) * 384:(lA * DC + c + 1) * 384],
                            start=(c == 0), stop=(c == DC - 1))
                        nc.tensor.matmul(
                            ps[64:128, :],
                            lhsT=tokT_sb[:, c * R_MAX:(c + 1) * R_MAX],
                            rhs=wt[:, (lB * DC + c) * 384:(lB * DC + c + 1) * 384],
                            start=(c == 0), stop=(c == DC - 1))
                    si, col = ((0, pair) if pair < 12
                               else (1, pair - 12) if pair < 23
                               else (2, pair - 23))
                    z = zp.tile([128, 384], F32, tag="z")
                    # u[:, pair] = sum_e (ps/SCALE) * dep  (fused mul+reduce)
                    nc.vector.scalar_tensor_tensor(
                        out=z[:], in0=ps[:], scalar=1.0 / SCALE, in1=dep_sb[:],
                        op0=OP.mult, op1=OP.mult,
                        accum_out=useg[si][:, col:col + 1])
                    pair += 1
                    for k, (lo, hi) in enumerate(U_SEGS):
                        if pair == hi:
                            nc.scalar.dma_start(out=u_d[:, lo:hi],
                                                in_=useg[k][:])

    nc.compile()
    _nc_cache["nc"] = nc
    return nc


def _shard_wc(Wc):
    """Per-core Wc layout: [128(p), OC*DC*384] fp8e4 of SCALE*Wc, with
    d = c*128 + p and free index f = (o_local*DC + c)*384 + e."""
    shards = []
    for k in range(NCORES):
        wck = Wc[k * OC:(k + 1) * OC]                  # [48, 384, 384]
        wck = wck.reshape(OC, DC, 128, 384)            # o, c, p, e
        wck = wck.transpose(2, 0, 1, 3)                # p, o, c, e
        q = (wck.reshape(128, OC * DC * 384) * SCALE).astype(
            ml_dtypes.float8_e4m3)
        shards.append(np.ascontiguousarray(q))
    return shards


def run_device(in_maps, trace=False, tmpdir=None):
    nc = _build_nc()
    return run_bass_kernel_spmd(nc, in_maps, list(range(NCORES)),
                                trace=trace, tmpdir=tmpdir)


def _make_in_maps(tok_sel, w_sel, wc_shards, bc):
    """tok_sel [R_MAX, D] f32 (w_sel/bc handled host-side post-epilogue)."""
    # tokT[p, c*R_MAX + r] = tok_sel[r, c*128 + p]
    tokT = np.ascontiguousarray(
        tok_sel.T.reshape(DC, 128, R_MAX).transpose(1, 0, 2)
    ).reshape(128, DC * R_MAX).astype(ml_dtypes.bfloat16)
    dep2 = np.tanh(np.concatenate([tok_sel, tok_sel], axis=0)).astype(np.float32)
    return [{"wc": wc_shards[k], "tokT": tokT, "dep": dep2}
            for k in range(NCORES)]


def kernel(**inputs):
    tokens = np.asarray(inputs["tokens"])
    heads = np.asarray(inputs["dep_heads"])
    tok_table = np.asarray(inputs["tok_table"], dtype=np.float32)
    Wc = np.asarray(inputs["Wc"], dtype=np.float32)
    bc = np.asarray(inputs["bc"], dtype=np.float32)
    Wr = np.asarray(inputs["Wr"], dtype=np.float32)
    br = np.asarray(inputs["br"], dtype=np.float32)
    assert tokens.shape == (B, S) and Wc.shape == (D, D, D)

    # host index selection: rows that can reach an unmasked (head==0) output row
    zs = [np.nonzero(heads[b] == 0)[0] for b in range(B)]
    sel = [(b, int(s2), int(heads[b, s2]))
           for b in range(B)
           for s2 in np.nonzero(np.isin(heads[b], zs[b]))[0]]
    R = len(sel)

    wc_shards = _shard_wc(Wc)
    w_full = Wr[0]
    toff = np.tanh(bc)

    contribs = []
    for lo in range(0, max(R, 1), R_MAX):
        chunk = sel[lo:lo + R_MAX]
        tok_sel = np.zeros((R_MAX, D), dtype=np.float32)
        w_sel = np.zeros(R_MAX, dtype=np.float32)
        for i, (b, s2, _dest) in enumerate(chunk):
            tok_sel[i] = tok_table[tokens[b, s2]]
            w_sel[i] = w_full[s2]
        res = run_device(_make_in_maps(tok_sel, w_sel, wc_shards, bc)).results
        # unscramble pair layout: rows 0-63 = even channels, 64-127 = odd
        parts = []
        for k in range(NCORES):
            uk = np.empty((R_MAX, OC), dtype=np.float32)
            uk[:, 0::2] = res[k]["u"][0:R_MAX]
            uk[:, 1::2] = res[k]["u"][R_MAX:2 * R_MAX]
            parts.append(uk)
        u = np.concatenate(parts, axis=1)              # [R_MAX, D]
        contribs.append(w_sel[:, None] * (np.tanh(u + bc[None, :])
                                          - toff[None, :]))

    base = (toff * w_full.sum() + br[0]).astype(np.float32)
    out = np.zeros((B, S, D), dtype=np.float32)
    for b in range(B):
        out[b, zs[b]] = base
    for i, (b, _s2, dest) in enumerate(sel):
        out[b, dest] += contribs[i // R_MAX][i % R_MAX]
    return out


# revision 16
# speedup vs baseline: 1.2868x; 1.0017x over previous
"""Trainium2 Bass kernel for nn_Composer (gnn_message_passing).

Math (exact reformulation of the reference):
  out[b,s1,:] = (heads[b,s1]==0) * ( base + sum_{s2: heads[b,s2]==s1} w[s2]*(t_on[b,s2]-t_off) )
  t_on[b,s2]  = tanh(u[b,s2] + bc),  u[b,s2,o] = tok[b,s2] @ Wc[o] @ tanh(tok[b,s2])
  t_off       = tanh(bc),  base = t_off*sum(w) + br

Only rows s2 whose head lands on a row with head==0 contribute to the output,
so u is needed for a handful of rows (R ~ 4-16 of 4096). The unavoidable cost
is streaming the bilinear weight Wc once; with the loose output tolerance Wc
is quantized host-side to fp8e4 (scaled by 32, rescaled in the reduce), which
cuts HBM traffic 4x vs f32 (7.08 MB/core). Sharding: Wc split over the output
dim O=384 across 8 cores (48 channels each). Channels are processed in PAIRS:
two col-tiled matmuls (tile_position (0,0)/(0,64)) put channel 2j in PSUM
partitions 0-63 and channel 2j+1 in 64-127 concurrently, so the fused
multiply+reduce against dep on the vector engine handles two channels per op.
The device outputs the raw bilinear values u; the tiny tanh/scale epilogue
(24 values/partition) runs on the host, keeping the device-side tail to one
reduce and one small DMA. The host does index selection, fp8/bf16 conversion,
sharding, and the final scatter of the ~R result vectors into the zero output.
"""
import numpy as np
import ml_dtypes

import concourse.bass as bass
import concourse.bacc as bacc
import concourse.mybir as mybir
from concourse.tile import TileContext
from concourse.bass_utils import run_bass_kernel_spmd

F32 = mybir.dt.float32
BF16 = mybir.dt.bfloat16
FP8 = mybir.dt.float8e4

B, S, D = 8, 512, 384
NCORES = 8
OC = D // NCORES          # output channels per core = 48
NPAIR = OC // 2           # channel pairs per core = 24
DC = D // 128             # contraction chunks = 3
R_MAX = 64                # padded selected-row capacity per device run
SCALE = 32.0              # Wc is stored as fp8(SCALE*Wc); undone in the reduce
# Wc transfer groups, alternating between the two HWDGE rings: big groups
# first for stream efficiency, small tail groups so the last pairs' data
# arrives just-in-time. Real matmuls are delayed behind a warm-up prefix
# (below) until the stream is half delivered, after which the PE runs
# back-to-back with data always buffered -- continuously busy, so the HAM
# clock gate never re-throttles it to half rate mid-stream.
GROUP_SIZES = [8, 8, 8, 8, 6, 6, 2, 2]
assert sum(GROUP_SIZES) == OC and all(g % 2 == 0 for g in GROUP_SIZES)
N_GRP = len(GROUP_SIZES)
# u output segments (in pair columns): flushed after pairs 11, 22, 23 so only
# the last single-pair segment's DMA completion sits on the critical tail
U_SEGS = [(0, 12), (12, 23), (23, 24)]

_nc_cache = {}


def _build_nc():
    if "nc" in _nc_cache:
        return _nc_cache["nc"]
    nc = bacc.Bacc("TRN2", target_bir_lowering=False, debug=False)
    wc_d = nc.dram_tensor("wc", [128, OC * DC * 384], FP8, kind="ExternalInput")
    tokT_d = nc.dram_tensor("tokT", [128, DC * R_MAX], BF16, kind="ExternalInput")
    dep_d = nc.dram_tensor("dep", [128, D], F32, kind="ExternalInput")
    u_d = nc.dram_tensor("u", [128, NPAIR], F32, kind="ExternalOutput")

    OP = mybir.AluOpType

    offs = [sum(GROUP_SIZES[:g]) for g in range(N_GRP)]

    with TileContext(nc) as tc:
        with (
            tc.tile_pool(name="const", bufs=1) as cp,
            tc.tile_pool(name="wcp", bufs=2 * N_GRP) as wcp,
            tc.tile_pool(name="zp", bufs=8) as zp,
            tc.tile_pool(name="pp", bufs=4, space="PSUM") as pp,
            tc.tile_pool(name="ppw", bufs=1, space="PSUM") as ppw,
        ):
            # dep is first needed by the first reduce: it leads the scalar
            # HWDGE ring; wc group-halves then alternate across both rings.
            dep_sb = cp.tile([128, D], F32)
            nc.scalar.dma_start(out=dep_sb[:], in_=dep_d[:])

            wts = []
            tokT_sb = None
            for g in range(N_GRP):
                wt = wcp.tile([128, GROUP_SIZES[g] * DC * 384], FP8,
                              tag="wc", name=f"wt{g}")
                eng = nc.sync if g % 2 == 0 else nc.scalar
                eng.dma_start(
                    out=wt[:],
                    in_=wc_d[:, offs[g] * DC * 384:
                             (offs[g] + GROUP_SIZES[g]) * DC * 384])
                wts.append(wt)
                if g == 0:
                    # tokT rides the SP ring right behind wc group 0
                    tokT_sb = cp.tile([128, DC * R_MAX], BF16)
                    nc.sync.dma_start(out=tokT_sb[:], in_=tokT_d[:])

            # PE warm-up/pacing prefix: ~10us of dummy matmuls on zeroed
            # scratch. The first ~3.4us flip the HAM clock gate to 8/8; the
            # rest hold the PE busy until the wc stream is ~half delivered,
            # so the real pair matmuls that follow run back-to-back at full
            # clock with their data always already in SBUF.
            wl = cp.tile([128, R_MAX], BF16, tag="wl", name="wl")
            wr = cp.tile([128, 384], FP8, tag="wr", name="wr")
            nc.gpsimd.memset(wl[:], 0)
            nc.gpsimd.memset(wr[:], 0)
            wps = ppw.tile([128, 384], F32, tag="wps")
            for _ in range(40):
                nc.tensor.matmul(wps[0:64, :], lhsT=wl[:], rhs=wr[:],
                                 start=True, stop=True)

            useg = [cp.tile([128, hi - lo], F32, tag=f"u{lo}", name=f"u{lo}")
                    for lo, hi in U_SEGS]

            pair = 0
            for g in range(N_GRP):
                wt = wts[g]
                for i in range(GROUP_SIZES[g] // 2):
                    lA, lB = 2 * i, 2 * i + 1          # local channel idx
                    ps = pp.tile([128, 384], F32, tag="ps")
                    # interleave the two col-tiles so both array halves
                    # stream their moving operand concurrently
                    for c in range(DC):
                        nc.tensor.matmul(
                            ps[0:64, :],
                            lhsT=tokT_sb[:, c * R_MAX:(c + 1) * R_MAX],
                            rhs=wt[:, (lA * DC + c) * 384:(lA * DC + c + 1) * 384],
                            start=(c == 0), stop=(c == DC - 1))
                        nc.tensor.matmul(
                            ps[64:128, :],
                            lhsT=tokT_sb[:, c * R_MAX:(c + 1) * R_MAX],
                            rhs=wt[:, (lB * DC + c) * 384:(lB * DC + c + 1) * 384],
                            start=(c == 0), stop=(c == DC - 1))
                    si, col = ((0, pair) if pair < 12
                               else (1, pair - 12) if pair < 23
                               else (2, pair - 23))
                    z = zp.tile([128, 384], F32, tag="z")
                    # u[:, pair] = sum_e (ps/SCALE) * dep  (fused mul+reduce)
                    nc.vector.scalar_tensor_tensor(
                        out=z[:], in0=ps[:], scalar=1.0 / SCALE, in1=dep_sb[:],
                        op0=OP.mult, op1=OP.mult,
                        accum_out=useg[si][:, col:col + 1])
                    pair += 1
                    for k, (lo, hi) in enumerate(U_SEGS):
                        if pair == hi:
                            nc.scalar.dma_start(out=u_d[:, lo:hi],
                                                in_=useg[k][:])

    nc.compile()
    _nc_cache["nc"] = nc
    return nc


def _shard_wc(Wc):
    """Per-core Wc layout: [128(p), OC*DC*384] fp8e4 of SCALE*Wc, with
    d = c*128 + p and free index f = (o_local*DC + c)*384 + e."""
    shards = []
    for k in range(NCORES):
        wck = Wc[k * OC:(k + 1) * OC]                  # [48, 384, 384]
        wck = wck.reshape(OC, DC, 128, 384)            # o, c, p, e
        wck = wck.transpose(2, 0, 1, 3)                # p, o, c, e
        q = (wck.reshape(128, OC * DC * 384) * SCALE).astype(
            ml_dtypes.float8_e4m3)
        shards.append(np.ascontiguousarray(q))
    return shards


def run_device(in_maps, trace=False, tmpdir=None):
    nc = _build_nc()
    return run_bass_kernel_spmd(nc, in_maps, list(range(NCORES)),
                                trace=trace, tmpdir=tmpdir)


def _make_in_maps(tok_sel, w_sel, wc_shards, bc):
    """tok_sel [R_MAX, D] f32 (w_sel/bc handled host-side post-epilogue)."""
    # tokT[p, c*R_MAX + r] = tok_sel[r, c*128 + p]
    tokT = np.ascontiguousarray(
        tok_sel.T.reshape(DC, 128, R_MAX).transpose(1, 0, 2)
    ).reshape(128, DC * R_MAX).astype(ml_dtypes.bfloat16)
    dep2 = np.tanh(np.concatenate([tok_sel, tok_sel], axis=0)).astype(np.float32)
    return [{"wc": wc_shards[k], "tokT": tokT, "dep": dep2}
            for k in range(NCORES)]


def kernel(**inputs):
    tokens = np.asarray(inputs["tokens"])
    heads = np.asarray(inputs["dep_heads"])
    tok_table = np.asarray(inputs["tok_table"], dtype=np.float32)
    Wc = np.asarray(inputs["Wc"], dtype=np.float32)
    bc = np.asarray(inputs["bc"], dtype=np.float32)
    Wr = np.asarray(inputs["Wr"], dtype=np.float32)
    br = np.asarray(inputs["br"], dtype=np.float32)
    assert tokens.shape == (B, S) and Wc.shape == (D, D, D)

    # host index selection: rows that can reach an unmasked (head==0) output row
    zs = [np.nonzero(heads[b] == 0)[0] for b in range(B)]
    sel = [(b, int(s2), int(heads[b, s2]))
           for b in range(B)
           for s2 in np.nonzero(np.isin(heads[b], zs[b]))[0]]
    R = len(sel)

    wc_shards = _shard_wc(Wc)
    w_full = Wr[0]
    toff = np.tanh(bc)

    contribs = []
    for lo in range(0, max(R, 1), R_MAX):
        chunk = sel[lo:lo + R_MAX]
        tok_sel = np.zeros((R_MAX, D), dtype=np.float32)
        w_sel = np.zeros(R_MAX, dtype=np.float32)
        for i, (b, s2, _dest) in enumerate(chunk):
            tok_sel[i] = tok_table[tokens[b, s2]]
            w_sel[i] = w_full[s2]
        res = run_device(_make_in_maps(tok_sel, w_sel, wc_shards, bc)).results
        # unscramble pair layout: rows 0-63 = even channels, 64-127 = odd
        parts = []
        for k in range(NCORES):
            uk = np.empty((R_MAX, OC), dtype=np.float32)
            uk[:, 0::2] = res[k]["u"][0:R_MAX]
            uk[:, 1::2] = res[k]["u"][R_MAX:2 * R_MAX]
            parts.append(uk)
        u = np.concatenate(parts, axis=1)              # [R_MAX, D]
        contribs.append(w_sel[:, None] * (np.tanh(u + bc[None, :])
                                          - toff[None, :]))

    base = (toff * w_full.sum() + br[0]).astype(np.float32)
    out = np.zeros((B, S, D), dtype=np.float32)
    for b in range(B):
        out[b, zs[b]] = base
    for i, (b, _s2, dest) in enumerate(sel):
        out[b, dest] += contribs[i // R_MAX][i % R_MAX]
    return out
